# revision 42
# baseline (speedup 1.0000x reference)
"""Distributed Trainium2 Bass kernel for ArcticAttention (GQA + RoPE + sliding window).

Sharding: tensor-parallel over heads across 8 cores. Core c owns q heads
4c..4c+3 and kv head c (exactly one GQA group). Per core:
  - q/k/v projections (bf16 matmuls, fp32 PSUM) producing qT/kT [dh, tok]
    and v [tok, dh] layouts,
  - RoPE fused on the vector engine from host-precomputed cos/sign-folded-sin
    tables,
  - sliding-window attention in S^T = K@Q^T layout (softmax over the
    partition axis via a ones-vector matmul; 1/l broadcast via gpsimd
    partition_broadcast; PV matmul needs no transposes anywhere),
  - AllGather of ctx^T features (bf16, 1 MB/core per batch-half, 4 total,
    interleaved with compute so the wire time hides),
  - column-sharded o_proj producing out^T [oc, tok]; host concatenates.
"""

import os
import sys

sys.path.insert(0, "/opt/pypackages")
sys.path.insert(0, "/opt/trn_rl_repo")

import numpy as np
import ml_dtypes

BF16 = ml_dtypes.bfloat16

B, S, HID = 2, 2048, 4096
H, HKV, DH = 32, 8, 128
G = H // HKV
WIN = 1024
THETA = 10000.0
NCORES = 8
HPC = H // NCORES          # 4 q heads per core
BT = B * S                 # 4096 tokens
QB = 512                   # token block for projections, attention, o_proj
NQB = S // QB              # 4 blocks per batch
NA = HID // 128            # 32 hid chunks
SCALE = 1.0 / float(np.sqrt(DH))

MASK_RS = (0, 1, 2, 3, 8, 9, 10, 11)
MSLOT = {r: i for i, r in enumerate(MASK_RS)}


def _span(r):
    qlo = max(0, (r - 8) * 128)
    qhi = min(QB, (r - 8) * 128 + 1024 + 127)
    return qlo, qhi


def _build_nc():
    import concourse.bass as bass
    import concourse.bacc as bacc
    import concourse.mybir as mybir
    from concourse import tile

    dt = mybir.dt
    bf = dt.bfloat16
    f32 = dt.float32
    AF = mybir.ActivationFunctionType

    nc = bacc.Bacc(
        "TRN2",
        target_bir_lowering=False,
        debug=False,
        enable_asserts=False,
        num_devices=NCORES,
    )

    # hidden, pre-tiled on host as [tb, p, a, t]: per token-block each SBUF
    # partition's data (all 32 a-chunks) is one contiguous 32KB DRAM run, so
    # the loads stream at full rate with a plain 2D access pattern.
    hiddenT = nc.dram_tensor("hiddenT", [(BT // QB) * 128, NA * QB], bf,
                             kind="ExternalInput")
    wq = nc.dram_tensor("wq", [HID, HPC * DH], bf, kind="ExternalInput")
    wk = nc.dram_tensor("wk", [HID, DH], bf, kind="ExternalInput")
    wv = nc.dram_tensor("wv", [HID, DH], bf, kind="ExternalInput")
    wo = nc.dram_tensor("wo", [HID, HPC * DH], bf, kind="ExternalInput")
    cost = nc.dram_tensor("cost", [DH, S], f32, kind="ExternalInput")
    sinm = nc.dram_tensor("sinm", [DH, S], f32, kind="ExternalInput")
    maskt = nc.dram_tensor("maskt", [len(MASK_RS) * 128, QB], bf, kind="ExternalInput")
    ident = nc.dram_tensor("ident", [128, 128], bf, kind="ExternalInput")
    outT = nc.dram_tensor("outT", [HPC * DH, BT], f32, kind="ExternalOutput")

    wq3 = wq[:].rearrange("(a p) d -> p a d", p=128)
    wk3 = wk[:].rearrange("(a p) d -> p a d", p=128)
    wv3 = wv[:].rearrange("(a p) d -> p a d", p=128)
    wo3 = wo[:].rearrange("(a p) d -> p a d", p=128)
    mask3 = maskt[:].rearrange("(m p) q -> p m q", p=128)

    with tile.TileContext(nc) as tc:
        with (
            tc.tile_pool(name="const", bufs=1) as cpool,
            tc.tile_pool(name="hid", bufs=3) as hidpool,
            tc.tile_pool(name="kv", bufs=2) as kvpool,
            tc.tile_pool(name="qt", bufs=5) as qtpool,
            tc.tile_pool(name="work", bufs=2) as wpool,
            tc.tile_pool(name="pt", bufs=3) as ptpool,
            tc.tile_pool(name="mm", bufs=3, space="PSUM") as mmpool,
            tc.tile_pool(name="sps", bufs=2, space="PSUM") as spool,
            tc.tile_pool(name="ctxps", bufs=1, space="PSUM") as cxpool,
            tc.tile_pool(name="lps", bufs=1, space="PSUM") as lpool,
            tc.tile_pool(name="tp", bufs=1, space="PSUM") as tppool,
            tc.tile_pool(name="dram", bufs=1, space="DRAM") as dpool,
        ):
            # ---- resident constants (single batched DMAs) ----
            wq_sb = cpool.tile([128, NA * HPC * DH], bf, tag="wq")
            wk_sb = cpool.tile([128, NA * DH], bf, tag="wk")
            wv_sb = cpool.tile([128, NA * DH], bf, tag="wv")
            wo_sb = cpool.tile([128, NA * HPC * DH], bf, tag="wo")
            cos_sb = cpool.tile([128, S], f32, tag="cos")
            sin_sb = cpool.tile([128, S], f32, tag="sin")
            mask_sb = cpool.tile([128, len(MASK_RS) * QB], bf, tag="mask")
            ones_sb = cpool.tile([128, 1], bf, tag="ones")
            id_sb = cpool.tile([128, 128], bf, tag="ident")

            # Spread preloads across engine DMA queues so the first matmuls
            # (needing wq half 0 on scalar + the first hidden block on sync)
            # start ~20us in instead of waiting on 12MB of serial loads.
            NH = NA // 2
            nc.scalar.dma_start(
                wq_sb[:, : NH * 512].rearrange("p (a d) -> p a d", a=NH),
                wq3[:, :NH, :],
            )
            nc.gpsimd.dma_start(
                wq_sb[:, NH * 512 :].rearrange("p (a d) -> p a d", a=NH),
                wq3[:, NH:, :],
            )
            nc.scalar.dma_start(cos_sb[:], cost[:])
            nc.scalar.dma_start(sin_sb[:], sinm[:])
            nc.gpsimd.dma_start(
                wk_sb[:].rearrange("p (a d) -> p a d", a=NA), wk3[:, :, :]
            )
            nc.gpsimd.dma_start(
                wv_sb[:].rearrange("p (a d) -> p a d", a=NA), wv3[:, :, :]
            )
            nc.gpsimd.dma_start(
                mask_sb[:].rearrange("p (m q) -> p m q", m=len(MASK_RS)),
                mask3[:, :, :],
            )
            nc.gpsimd.dma_start(id_sb[:], ident[:])
            nc.gpsimd.dma_start(
                wo_sb[:].rearrange("p (a d) -> p a d", a=NA), wo3[:, :, :]
            )
            nc.any.memset(ones_sb[:], 1.0)

            # per (batch, tok-half) collective bounce buffers
            ctxl = [
                [
                    dpool.tile(
                        [HPC * DH, S // 2], bf,
                        tag=f"ctxl{b}{hf}", name=f"ctxl{b}{hf}",
                    )
                    for hf in range(2)
                ]
                for b in range(B)
            ]
            ctxf = [
                [
                    dpool.tile(
                        [H * DH, S // 2], bf, addr_space="Shared",
                        tag=f"ctxf{b}{hf}", name=f"ctxf{b}{hf}",
                    )
                    for hf in range(2)
                ]
                for b in range(B)
            ]

            def load_half(src3, gofs, a0, n, width=QB):
                """One DMA: chunks [a0, a0+n) of a (a p)-major DRAM tensor into
                an SBUF tile laid out [128, n*width]."""
                t = hidpool.tile([128, n * width], bf, tag="hid", name=f"hid{gofs}_{a0}")
                nc.sync.dma_start(
                    t[:].rearrange("p (a t) -> p a t", a=n),
                    src3[:, a0 : a0 + n, gofs : gofs + width],
                )
                return t

            def load_hid_half(tb, a0, n):
                """Contiguous pre-tiled hidden load for token block tb."""
                t = hidpool.tile([128, n * QB], bf, tag="hid", name=f"hid{tb}_{a0}")
                nc.sync.dma_start(
                    t[:],
                    hiddenT[tb * 128 : (tb + 1) * 128,
                            a0 * QB : (a0 + n) * QB],
                )
                return t

            def rope_drain(ps, dst, tok0):
                """dst(bf16) = ps * cos + rot_half(ps) * sin (sign-folded)."""
                t1 = wpool.tile([128, QB], f32, tag="ropet1")
                t2 = wpool.tile([128, QB], f32, tag="ropet2")
                cs = cos_sb[:, tok0 : tok0 + QB]
                sn = sin_sb[:, tok0 : tok0 + QB]
                nc.vector.tensor_mul(t1[:], ps, cs)
                nc.vector.tensor_mul(t2[0:64, :], ps[64:128, :], sn[0:64, :])
                nc.vector.tensor_mul(t2[64:128, :], ps[0:64, :], sn[64:128, :])
                nc.vector.tensor_add(dst, t1[:], t2[:])

            def proj_block(b, qbi, kT_sb, v_sb):
                """Projections + RoPE for tokens [qbi*QB, (qbi+1)*QB) of batch b.
                Returns the 4 per-head qT tiles."""
                ltok = qbi * QB
                tb = b * NQB + qbi
                halves = [load_hid_half(tb, 0, NA // 2),
                          load_hid_half(tb, NA // 2, NA // 2)]
                qts = [
                    qtpool.tile([128, QB], bf, tag="qtile", name=f"qt{b}_{qbi}_{h}")
                    for h in range(HPC)
                ]
                # group 1: q heads 0..2 ; group 2: q head 3, k, v
                # NOTE: start=True clears has_written for the whole PSUM bank,
                # so regions sharing a bank (v's 4 tok-subtiles) must each run
                # their full accumulation consecutively (j outer, a inner).
                for grp in (("q0", "q1", "q2"), ("q3", "k", "v")):
                    ps = {u: mmpool.tile([128, QB], f32, tag="mmps", name=f"ps{u}{b}{qbi}")
                          for u in grp}
                    for hf in range(2):
                        hs = halves[hf]
                        for u in grp:
                            if u == "v":
                                continue
                            for ai in range(NA // 2):
                                a = hf * (NA // 2) + ai
                                st = a == 0
                                sp = a == NA - 1
                                if u[0] == "q":
                                    h = int(u[1])
                                    nc.tensor.matmul(
                                        ps[u][:],
                                        wq_sb[:, a * 512 + h * 128 : a * 512 + (h + 1) * 128],
                                        hs[:, ai * QB : (ai + 1) * QB],
                                        start=st, stop=sp,
                                    )
                                else:
                                    nc.tensor.matmul(
                                        ps[u][:],
                                        wk_sb[:, a * 128 : (a + 1) * 128],
                                        hs[:, ai * QB : (ai + 1) * QB],
                                        start=st, stop=sp,
                                    )
                        if "v" in grp:
                            # vT [dh, tok] like k (N=512 matmuls), transposed
                            # to v [tok, dh] below via PE transpose-mode.
                            for ai in range(NA // 2):
                                a = hf * (NA // 2) + ai
                                nc.tensor.matmul(
                                    ps["v"][:],
                                    wv_sb[:, a * 128 : (a + 1) * 128],
                                    hs[:, ai * QB : (ai + 1) * QB],
                                    start=(a == 0), stop=(a == NA - 1),
                                )
                    for u in grp:
                        if u[0] == "q":
                            rope_drain(ps[u][:], qts[int(u[1])][:], ltok)
                        elif u == "k":
                            rope_drain(ps[u][:], kT_sb[:, ltok : ltok + QB], ltok)
                        else:
                            vt_sb = wpool.tile([128, QB], bf, tag="vtsb", name=f"vt{b}{qbi}", bufs=1)
                            nc.vector.tensor_copy(vt_sb[:], ps[u][:])
                            for j in range(4):
                                tp = tppool.tile([128, 128], bf, tag="tp", name=f"tp{b}{qbi}{j}")
                                nc.tensor.transpose(
                                    tp[:], vt_sb[:, j * 128 : (j + 1) * 128], id_sb[:]
                                )
                                nc.vector.tensor_copy(
                                    v_sb[:, ltok + j * 128 : ltok + (j + 1) * 128], tp[:]
                                )
                return qts

            def attn_block(b, qbi, qts, kT_sb, v_sb):
                Q0 = 4 * qbi
                kts = [Q0] + [kt for kt in range(max(0, Q0 - 8), Q0 + 4) if kt != Q0]
                for h in range(HPC):
                    qt = qts[h]
                    ctx_ps = cxpool.tile([128, QB], f32, tag="ctxps", name=f"cx{b}{qbi}{h}")
                    l_ps = lpool.tile([1, QB], f32, tag="lps", name=f"l{b}{qbi}{h}")
                    # two alternating f32 accumulators collapse the per-key-tile
                    # prob tiles on DVE; the partition-axis sum then needs only
                    # two ones-matmuls instead of one per key tile.
                    accs = [
                        wpool.tile([128, QB], f32, tag=f"lacc{p}", name=f"la{p}_{b}{qbi}{h}")
                        for p in range(2)
                    ]
                    acc_used = [False, False]
                    for idx, kt in enumerate(kts):
                        r = kt - (Q0 - 8)
                        qlo, qhi = _span(r)
                        s_ps = spool.tile([128, QB], f32, tag="sps", name=f"s{b}{qbi}{h}{kt}")
                        nc.tensor.matmul(
                            s_ps[:, qlo:qhi],
                            kT_sb[:, kt * 128 : (kt + 1) * 128],
                            qt[:, qlo:qhi],
                            start=True, stop=True,
                        )
                        pt = ptpool.tile([128, QB], bf, tag="pt", name=f"pt{b}{qbi}{h}{kt}")
                        nc.scalar.activation(
                            pt[:, qlo:qhi], s_ps[:, qlo:qhi], AF.Exp, scale=SCALE
                        )
                        if r in MSLOT:
                            m0 = MSLOT[r] * QB
                            nc.vector.tensor_mul(
                                pt[:, qlo:qhi],
                                pt[:, qlo:qhi],
                                mask_sb[:, m0 + qlo : m0 + qhi],
                            )
                        last = idx == len(kts) - 1
                        nc.tensor.matmul(
                            ctx_ps[:, qlo:qhi],
                            v_sb[:, kt * 128 : (kt + 1) * 128],
                            pt[:, qlo:qhi],
                            start=(idx == 0), stop=last,
                        )
                        par = idx % 2
                        if not acc_used[par]:
                            # first tile on this chain: plain copy (idx 0 and 1
                            # are the full-span kt=Q0 and a window tile; both
                            # chains start with a copy over their live span,
                            # but only idx 0 is guaranteed full span, so chain 1
                            # zero-fills first)
                            if idx == 0:
                                nc.vector.tensor_copy(accs[par][:], pt[:])
                            else:
                                nc.any.memset(accs[par][:], 0.0)
                                nc.vector.tensor_add(
                                    accs[par][:, qlo:qhi],
                                    accs[par][:, qlo:qhi],
                                    pt[:, qlo:qhi],
                                )
                            acc_used[par] = True
                        else:
                            nc.vector.tensor_add(
                                accs[par][:, qlo:qhi],
                                accs[par][:, qlo:qhi],
                                pt[:, qlo:qhi],
                            )
                    nparts = sum(acc_used)
                    for p in range(nparts):
                        accb = wpool.tile([128, QB], bf, tag=f"laccb{p}", name=f"lb{p}_{b}{qbi}{h}", bufs=1)
                        nc.vector.tensor_copy(accb[:], accs[p][:])
                        nc.tensor.matmul(
                            l_ps[0:1, :],
                            ones_sb[:, 0:1],
                            accb[:],
                            start=(p == 0), stop=(p == nparts - 1),
                        )
                    lrec = wpool.tile([1, QB], f32, tag="lrec", name=f"lr{b}{qbi}{h}", bufs=1)
                    nc.vector.reciprocal_approx_fast(lrec[:], l_ps[:])
                    lb = wpool.tile([128, QB], f32, tag="lb", name=f"lb{b}{qbi}{h}")
                    nc.gpsimd.partition_broadcast(lb[:], lrec[0:1, :])
                    ctx_sb = wpool.tile([128, QB], bf, tag="ctxsb", name=f"cs{b}{qbi}{h}")
                    nc.vector.tensor_mul(ctx_sb[:], ctx_ps[:], lb[:])

                    nc.sync.dma_start(
                        ctxl[b][qbi // 2][
                            h * 128 : (h + 1) * 128,
                            (qbi % 2) * QB : (qbi % 2 + 1) * QB,
                        ],
                        ctx_sb[:],
                    )

            def allgather(b, hf):
                nc.gpsimd.collective_compute(
                    "AllGather",
                    __import__("concourse.mybir", fromlist=["AluOpType"]).AluOpType.bypass,
                    replica_groups=[list(range(NCORES))],
                    ins=[ctxl[b][hf][:].opt()],
                    outs=[ctxf[b][hf][:].opt()],
                )

            def oproj_block(b, tbo):
                """out^T[oc, tok] for tokens [tbo*QB, +QB) of batch b."""
                ltok = tbo * QB
                gtok = b * S + ltok
                src3 = ctxf[b][tbo // 2][:].rearrange("(a p) t -> p a t", p=128)
                lofs = (tbo % 2) * QB
                cfs = []
                for hf in range(2):
                    t = hidpool.tile(
                        [128, (NA // 2) * QB], bf, tag="hid", name=f"cf{b}{tbo}{hf}"
                    )
                    nc.sync.dma_start(
                        t[:].rearrange("p (a t) -> p a t", a=NA // 2),
                        src3[:, hf * (NA // 2) : (hf + 1) * (NA // 2), lofs : lofs + QB],
                    )
                    cfs.append(t)
                for oc in range(HPC):
                    ps = mmpool.tile([128, QB], f32, tag="mmps", name=f"ops{b}{tbo}{oc}")
                    for a in range(NA):
                        nc.tensor.matmul(
                            ps[:],
                            wo_sb[:, a * 512 + oc * 128 : a * 512 + (oc + 1) * 128],
                            cfs[a // (NA // 2)][:, (a % (NA // 2)) * QB : (a % (NA // 2) + 1) * QB],
                            start=(a == 0), stop=(a == NA - 1),
                        )
                    osb = wpool.tile([128, QB], f32, tag="osb", name=f"ob{b}{tbo}{oc}")
                    nc.vector.tensor_copy(osb[:], ps[:])
                    nc.sync.dma_start(
                        outT[oc * 128 : (oc + 1) * 128, gtok : gtok + QB], osb[:]
                    )

            # ================= emission schedule =================
            for b in range(B):
                kT_sb = kvpool.tile([128, S], bf, tag="kT", name=f"kT{b}")
                v_sb = kvpool.tile([128, S], bf, tag="v", name=f"v{b}")
                for qbi in range(NQB):
                    qts = proj_block(b, qbi, kT_sb, v_sb)
                    attn_block(b, qbi, qts, kT_sb, v_sb)
                    if qbi == 1:
                        allgather(b, 0)
                    if b == 1 and qbi >= 2:
                        oproj_block(0, qbi)  # overlap b0 o_proj with b1 tail
                allgather(b, 1)
            oproj_block(0, 0)
            oproj_block(0, 1)
            for tbo in range(NQB):
                oproj_block(1, tbo)

    nc.compile()
    return nc


_NC = None


def _get_nc():
    global _NC
    if _NC is None:
        _NC = _build_nc()
    return _NC


def _prep_inputs(hidden_states, q_proj_w, k_proj_w, v_proj_w, o_proj_w, position_ids):
    hidden_states = np.asarray(hidden_states, dtype=np.float32)
    # pre-tile: hT[tb, p, a, t] = hidden[tb*QB + t, a*128 + p]
    hT = np.ascontiguousarray(
        hidden_states.reshape(BT // QB, QB, NA, 128).transpose(0, 3, 2, 1)
    ).astype(BF16).reshape((BT // QB) * 128, NA * QB)

    pos = np.asarray(position_ids)[0].astype(np.float32)  # [S]
    inv = 1.0 / (THETA ** (np.arange(0, DH, 2, dtype=np.float32) / DH))  # [64]
    ang = pos[:, None] * inv[None, :]  # [S, 64]
    c = np.cos(ang).T.astype(np.float32)  # [64, S]
    s = np.sin(ang).T.astype(np.float32)
    cost = np.ascontiguousarray(np.concatenate([c, c], axis=0))
    sinm = np.ascontiguousarray(np.concatenate([-s, s], axis=0))

    kj = np.arange(128)[:, None]
    qi = np.arange(QB)[None, :]
    masks = []
    for r in MASK_RS:
        d = (8 - r) * 128 + qi - kj
        masks.append(((d >= 0) & (d < WIN)).astype(np.float32))
    maskt = np.ascontiguousarray(np.concatenate(masks, axis=0)).astype(BF16)

    q_proj_w = np.asarray(q_proj_w, dtype=np.float32)
    k_proj_w = np.asarray(k_proj_w, dtype=np.float32)
    v_proj_w = np.asarray(v_proj_w, dtype=np.float32)
    o_proj_w = np.asarray(o_proj_w, dtype=np.float32)

    in_maps = []
    for core in range(NCORES):
        r0q = core * HPC * DH
        r0k = core * DH
        in_maps.append(
            {
                "hiddenT": hT,
                "wq": np.ascontiguousarray(
                    q_proj_w[r0q : r0q + HPC * DH, :].T
                ).astype(BF16),
                "wk": np.ascontiguousarray(k_proj_w[r0k : r0k + DH, :].T).astype(BF16),
                "wv": np.ascontiguousarray(v_proj_w[r0k : r0k + DH, :].T).astype(BF16),
                "wo": np.ascontiguousarray(
                    o_proj_w[r0q : r0q + HPC * DH, :].T
                ).astype(BF16),
                "cost": cost,
                "sinm": sinm,
                "maskt": maskt,
                "ident": np.eye(128, dtype=np.float32).astype(BF16),
            }
        )
    return in_maps


def run(inputs, trace=False):
    from concourse.bass_utils import run_bass_kernel_spmd

    nc = _get_nc()
    in_maps = _prep_inputs(
        inputs["hidden_states"],
        inputs["q_proj_w"],
        inputs["k_proj_w"],
        inputs["v_proj_w"],
        inputs["o_proj_w"],
        inputs["position_ids"],
    )
    res = run_bass_kernel_spmd(
        nc, in_maps, core_ids=list(range(NCORES)), trace=trace
    )
    out = np.empty((BT, HID), dtype=np.float32)
    for core in range(NCORES):
        o = np.asarray(res.results[core]["outT"], dtype=np.float32)  # [512, BT]
        out[:, core * HPC * DH : (core + 1) * HPC * DH] = o.T
    return out.reshape(B, S, HID), res


def kernel(**inputs):
    out, _ = run(inputs, trace=False)
    return out


# revision 45
# speedup vs baseline: 1.0529x; 1.0529x over previous
"""Distributed Trainium2 Bass kernel for ArcticAttention (GQA + RoPE + sliding window).

Sharding: tensor-parallel over heads across 8 cores. Core c owns q heads
4c..4c+3 and kv head c (exactly one GQA group). Per core:
  - q/k/v projections (bf16 matmuls, fp32 PSUM) producing qT/kT [dh, tok]
    and v [tok, dh] layouts,
  - RoPE fused on the vector engine from host-precomputed cos/sign-folded-sin
    tables,
  - sliding-window attention in S^T = K@Q^T layout (softmax over the
    partition axis via a ones-vector matmul; 1/l broadcast via gpsimd
    partition_broadcast; PV matmul needs no transposes anywhere),
  - AllGather of ctx^T features (bf16, 1 MB/core per batch-half, 4 total,
    interleaved with compute so the wire time hides),
  - column-sharded o_proj producing out^T [oc, tok]; host concatenates.
"""

import os
import sys

sys.path.insert(0, "/opt/pypackages")
sys.path.insert(0, "/opt/trn_rl_repo")

import numpy as np
import ml_dtypes

BF16 = ml_dtypes.bfloat16

B, S, HID = 2, 2048, 4096
H, HKV, DH = 32, 8, 128
G = H // HKV
WIN = 1024
THETA = 10000.0
NCORES = 8
HPC = H // NCORES          # 4 q heads per core
BT = B * S                 # 4096 tokens
QB = 512                   # token block for projections, attention, o_proj
NQB = S // QB              # 4 blocks per batch
NA = HID // 128            # 32 hid chunks
SCALE = 1.0 / float(np.sqrt(DH))

MASK_RS = (0, 1, 2, 3, 8, 9, 10, 11)
MSLOT = {r: i for i, r in enumerate(MASK_RS)}


def _span(r):
    qlo = max(0, (r - 8) * 128)
    qhi = min(QB, (r - 8) * 128 + 1024 + 127)
    return qlo, qhi


def _build_nc():
    import concourse.bass as bass
    import concourse.bacc as bacc
    import concourse.mybir as mybir
    from concourse import tile

    dt = mybir.dt
    bf = dt.bfloat16
    f32 = dt.float32
    AF = mybir.ActivationFunctionType

    nc = bacc.Bacc(
        "TRN2",
        target_bir_lowering=False,
        debug=False,
        enable_asserts=False,
        num_devices=NCORES,
    )

    # hidden, pre-tiled on host as [tb, p, a, t]: per token-block each SBUF
    # partition's data (all 32 a-chunks) is one contiguous 32KB DRAM run, so
    # the loads stream at full rate with a plain 2D access pattern.
    hiddenT = nc.dram_tensor("hiddenT", [(BT // QB) * 128, NA * QB], bf,
                             kind="ExternalInput")
    # weights/masks host-prearranged to [128, chunks*width] matching the SBUF
    # tile layout exactly -> plain contiguous 2D DMAs
    wq = nc.dram_tensor("wq", [128, NA * HPC * DH], bf, kind="ExternalInput")
    wk = nc.dram_tensor("wk", [128, NA * DH], bf, kind="ExternalInput")
    wv = nc.dram_tensor("wv", [128, NA * DH], bf, kind="ExternalInput")
    wo = nc.dram_tensor("wo", [128, NA * HPC * DH], bf, kind="ExternalInput")
    cost = nc.dram_tensor("cost", [DH, S], f32, kind="ExternalInput")
    sinm = nc.dram_tensor("sinm", [DH, S], f32, kind="ExternalInput")
    maskt = nc.dram_tensor("maskt", [128, len(MASK_RS) * QB], bf, kind="ExternalInput")
    ident = nc.dram_tensor("ident", [128, 128], bf, kind="ExternalInput")
    outT = nc.dram_tensor("outT", [HPC * DH, BT], f32, kind="ExternalOutput")

    with tile.TileContext(nc) as tc:
        with (
            tc.tile_pool(name="const", bufs=1) as cpool,
            tc.tile_pool(name="hid", bufs=3) as hidpool,
            tc.tile_pool(name="kv", bufs=2) as kvpool,
            tc.tile_pool(name="qt", bufs=5) as qtpool,
            tc.tile_pool(name="work", bufs=2) as wpool,
            tc.tile_pool(name="pt", bufs=3) as ptpool,
            tc.tile_pool(name="mm", bufs=3, space="PSUM") as mmpool,
            tc.tile_pool(name="sps", bufs=2, space="PSUM") as spool,
            tc.tile_pool(name="ctxps", bufs=1, space="PSUM") as cxpool,
            tc.tile_pool(name="lps", bufs=1, space="PSUM") as lpool,
            tc.tile_pool(name="tp", bufs=1, space="PSUM") as tppool,
            tc.tile_pool(name="dram", bufs=1, space="DRAM") as dpool,
        ):
            # ---- resident constants (single batched DMAs) ----
            wq_sb = cpool.tile([128, NA * HPC * DH], bf, tag="wq")
            wk_sb = cpool.tile([128, NA * DH], bf, tag="wk")
            wv_sb = cpool.tile([128, NA * DH], bf, tag="wv")
            wo_sb = cpool.tile([128, NA * HPC * DH], bf, tag="wo")
            cos_sb = cpool.tile([128, S], f32, tag="cos")
            sin_sb = cpool.tile([128, S], f32, tag="sin")
            mask_sb = cpool.tile([128, len(MASK_RS) * QB], bf, tag="mask")
            ones_sb = cpool.tile([128, 1], bf, tag="ones")
            id_sb = cpool.tile([128, 128], bf, tag="ident")

            # Spread preloads across engine DMA queues so the first matmuls
            # (needing wq half 0 on scalar + the first hidden block on sync)
            # start ~20us in instead of waiting on 12MB of serial loads.
            NH = NA // 2
            nc.scalar.dma_start(wq_sb[:, : NH * 512], wq[:, : NH * 512])
            nc.gpsimd.dma_start(wq_sb[:, NH * 512 :], wq[:, NH * 512 :])
            nc.scalar.dma_start(cos_sb[:], cost[:])
            nc.scalar.dma_start(sin_sb[:], sinm[:])
            nc.gpsimd.dma_start(wk_sb[:], wk[:])
            nc.gpsimd.dma_start(wv_sb[:], wv[:])
            nc.gpsimd.dma_start(mask_sb[:], maskt[:])
            nc.gpsimd.dma_start(id_sb[:], ident[:])
            nc.gpsimd.dma_start(wo_sb[:], wo[:])
            nc.any.memset(ones_sb[:], 1.0)

            # per (batch, tok-half) collective bounce buffers
            ctxl = [
                [
                    dpool.tile(
                        [HPC * DH, S // 2], bf,
                        tag=f"ctxl{b}{hf}", name=f"ctxl{b}{hf}",
                    )
                    for hf in range(2)
                ]
                for b in range(B)
            ]
            ctxf = [
                [
                    dpool.tile(
                        [H * DH, S // 2], bf, addr_space="Shared",
                        tag=f"ctxf{b}{hf}", name=f"ctxf{b}{hf}",
                    )
                    for hf in range(2)
                ]
                for b in range(B)
            ]

            def load_half(src3, gofs, a0, n, width=QB):
                """One DMA: chunks [a0, a0+n) of a (a p)-major DRAM tensor into
                an SBUF tile laid out [128, n*width]."""
                t = hidpool.tile([128, n * width], bf, tag="hid", name=f"hid{gofs}_{a0}")
                nc.sync.dma_start(
                    t[:].rearrange("p (a t) -> p a t", a=n),
                    src3[:, a0 : a0 + n, gofs : gofs + width],
                )
                return t

            def load_hid_half(tb, a0, n):
                """Contiguous pre-tiled hidden load for token block tb."""
                t = hidpool.tile([128, n * QB], bf, tag="hid", name=f"hid{tb}_{a0}")
                nc.sync.dma_start(
                    t[:],
                    hiddenT[tb * 128 : (tb + 1) * 128,
                            a0 * QB : (a0 + n) * QB],
                )
                return t

            def rope_drain(ps, dst, tok0):
                """dst(bf16) = ps * cos + rot_half(ps) * sin (sign-folded)."""
                t1 = wpool.tile([128, QB], f32, tag="ropet1")
                t2 = wpool.tile([128, QB], f32, tag="ropet2")
                cs = cos_sb[:, tok0 : tok0 + QB]
                sn = sin_sb[:, tok0 : tok0 + QB]
                nc.vector.tensor_mul(t1[:], ps, cs)
                nc.vector.tensor_mul(t2[0:64, :], ps[64:128, :], sn[0:64, :])
                nc.vector.tensor_mul(t2[64:128, :], ps[0:64, :], sn[64:128, :])
                nc.vector.tensor_add(dst, t1[:], t2[:])

            def proj_block(b, qbi, kT_sb, v_sb):
                """Projections + RoPE for tokens [qbi*QB, (qbi+1)*QB) of batch b.
                Returns the 4 per-head qT tiles."""
                ltok = qbi * QB
                tb = b * NQB + qbi
                halves = [load_hid_half(tb, 0, NA // 2),
                          load_hid_half(tb, NA // 2, NA // 2)]
                qts = [
                    qtpool.tile([128, QB], bf, tag="qtile", name=f"qt{b}_{qbi}_{h}")
                    for h in range(HPC)
                ]
                # group 1: q heads 0..2 ; group 2: q head 3, k, v
                # NOTE: start=True clears has_written for the whole PSUM bank,
                # so regions sharing a bank (v's 4 tok-subtiles) must each run
                # their full accumulation consecutively (j outer, a inner).
                for grp in (("q0", "q1", "q2"), ("q3", "k", "v")):
                    ps = {u: mmpool.tile([128, QB], f32, tag="mmps", name=f"ps{u}{b}{qbi}")
                          for u in grp}
                    for hf in range(2):
                        hs = halves[hf]
                        for u in grp:
                            if u == "v":
                                continue
                            for ai in range(NA // 2):
                                a = hf * (NA // 2) + ai
                                st = a == 0
                                sp = a == NA - 1
                                if u[0] == "q":
                                    h = int(u[1])
                                    nc.tensor.matmul(
                                        ps[u][:],
                                        wq_sb[:, a * 512 + h * 128 : a * 512 + (h + 1) * 128],
                                        hs[:, ai * QB : (ai + 1) * QB],
                                        start=st, stop=sp,
                                    )
                                else:
                                    nc.tensor.matmul(
                                        ps[u][:],
                                        wk_sb[:, a * 128 : (a + 1) * 128],
                                        hs[:, ai * QB : (ai + 1) * QB],
                                        start=st, stop=sp,
                                    )
                        if "v" in grp:
                            # vT [dh, tok] like k (N=512 matmuls), transposed
                            # to v [tok, dh] below via PE transpose-mode.
                            for ai in range(NA // 2):
                                a = hf * (NA // 2) + ai
                                nc.tensor.matmul(
                                    ps["v"][:],
                                    wv_sb[:, a * 128 : (a + 1) * 128],
                                    hs[:, ai * QB : (ai + 1) * QB],
                                    start=(a == 0), stop=(a == NA - 1),
                                )
                    for u in grp:
                        if u[0] == "q":
                            rope_drain(ps[u][:], qts[int(u[1])][:], ltok)
                        elif u == "k":
                            rope_drain(ps[u][:], kT_sb[:, ltok : ltok + QB], ltok)
                        else:
                            vt_sb = wpool.tile([128, QB], bf, tag="vtsb", name=f"vt{b}{qbi}", bufs=1)
                            nc.vector.tensor_copy(vt_sb[:], ps[u][:])
                            for j in range(4):
                                tp = tppool.tile([128, 128], bf, tag="tp", name=f"tp{b}{qbi}{j}")
                                nc.tensor.transpose(
                                    tp[:], vt_sb[:, j * 128 : (j + 1) * 128], id_sb[:]
                                )
                                nc.vector.tensor_copy(
                                    v_sb[:, ltok + j * 128 : ltok + (j + 1) * 128], tp[:]
                                )
                return qts

            def attn_block(b, qbi, qts, kT_sb, v_sb):
                Q0 = 4 * qbi
                kts = [Q0] + [kt for kt in range(max(0, Q0 - 8), Q0 + 4) if kt != Q0]
                for h in range(HPC):
                    qt = qts[h]
                    ctx_ps = cxpool.tile([128, QB], f32, tag="ctxps", name=f"cx{b}{qbi}{h}")
                    l_ps = lpool.tile([1, QB], f32, tag="lps", name=f"l{b}{qbi}{h}")
                    # two alternating f32 accumulators collapse the per-key-tile
                    # prob tiles on DVE; the partition-axis sum then needs only
                    # two ones-matmuls instead of one per key tile.
                    accs = [
                        wpool.tile([128, QB], f32, tag=f"lacc{p}", name=f"la{p}_{b}{qbi}{h}")
                        for p in range(2)
                    ]
                    acc_used = [False, False]
                    for idx, kt in enumerate(kts):
                        r = kt - (Q0 - 8)
                        qlo, qhi = _span(r)
                        s_ps = spool.tile([128, QB], f32, tag="sps", name=f"s{b}{qbi}{h}{kt}")
                        nc.tensor.matmul(
                            s_ps[:, qlo:qhi],
                            kT_sb[:, kt * 128 : (kt + 1) * 128],
                            qt[:, qlo:qhi],
                            start=True, stop=True,
                        )
                        pt = ptpool.tile([128, QB], bf, tag="pt", name=f"pt{b}{qbi}{h}{kt}")
                        nc.scalar.activation(
                            pt[:, qlo:qhi], s_ps[:, qlo:qhi], AF.Exp, scale=SCALE
                        )
                        if r in MSLOT:
                            m0 = MSLOT[r] * QB
                            nc.vector.tensor_mul(
                                pt[:, qlo:qhi],
                                pt[:, qlo:qhi],
                                mask_sb[:, m0 + qlo : m0 + qhi],
                            )
                        last = idx == len(kts) - 1
                        nc.tensor.matmul(
                            ctx_ps[:, qlo:qhi],
                            v_sb[:, kt * 128 : (kt + 1) * 128],
                            pt[:, qlo:qhi],
                            start=(idx == 0), stop=last,
                        )
                        par = idx % 2
                        if not acc_used[par]:
                            # first tile on this chain: plain copy (idx 0 and 1
                            # are the full-span kt=Q0 and a window tile; both
                            # chains start with a copy over their live span,
                            # but only idx 0 is guaranteed full span, so chain 1
                            # zero-fills first)
                            if idx == 0:
                                nc.vector.tensor_copy(accs[par][:], pt[:])
                            else:
                                nc.any.memset(accs[par][:], 0.0)
                                nc.vector.tensor_add(
                                    accs[par][:, qlo:qhi],
                                    accs[par][:, qlo:qhi],
                                    pt[:, qlo:qhi],
                                )
                            acc_used[par] = True
                        else:
                            nc.vector.tensor_add(
                                accs[par][:, qlo:qhi],
                                accs[par][:, qlo:qhi],
                                pt[:, qlo:qhi],
                            )
                    nparts = sum(acc_used)
                    for p in range(nparts):
                        accb = wpool.tile([128, QB], bf, tag=f"laccb{p}", name=f"lb{p}_{b}{qbi}{h}", bufs=1)
                        nc.vector.tensor_copy(accb[:], accs[p][:])
                        nc.tensor.matmul(
                            l_ps[0:1, :],
                            ones_sb[:, 0:1],
                            accb[:],
                            start=(p == 0), stop=(p == nparts - 1),
                        )
                    lrec = wpool.tile([1, QB], f32, tag="lrec", name=f"lr{b}{qbi}{h}", bufs=1)
                    nc.vector.reciprocal_approx_fast(lrec[:], l_ps[:])
                    lb = wpool.tile([128, QB], f32, tag="lb", name=f"lb{b}{qbi}{h}")
                    nc.gpsimd.partition_broadcast(lb[:], lrec[0:1, :])
                    ctx_sb = wpool.tile([128, QB], bf, tag="ctxsb", name=f"cs{b}{qbi}{h}")
                    nc.vector.tensor_mul(ctx_sb[:], ctx_ps[:], lb[:])

                    nc.sync.dma_start(
                        ctxl[b][qbi // 2][
                            h * 128 : (h + 1) * 128,
                            (qbi % 2) * QB : (qbi % 2 + 1) * QB,
                        ],
                        ctx_sb[:],
                    )

            def allgather(b, hf):
                nc.gpsimd.collective_compute(
                    "AllGather",
                    __import__("concourse.mybir", fromlist=["AluOpType"]).AluOpType.bypass,
                    replica_groups=[list(range(NCORES))],
                    ins=[ctxl[b][hf][:].opt()],
                    outs=[ctxf[b][hf][:].opt()],
                )

            def oproj_block(b, tbo):
                """out^T[oc, tok] for tokens [tbo*QB, +QB) of batch b."""
                ltok = tbo * QB
                gtok = b * S + ltok
                src3 = ctxf[b][tbo // 2][:].rearrange("(a p) t -> p a t", p=128)
                lofs = (tbo % 2) * QB
                cfs = []
                for hf in range(2):
                    t = hidpool.tile(
                        [128, (NA // 2) * QB], bf, tag="hid", name=f"cf{b}{tbo}{hf}"
                    )
                    nc.sync.dma_start(
                        t[:].rearrange("p (a t) -> p a t", a=NA // 2),
                        src3[:, hf * (NA // 2) : (hf + 1) * (NA // 2), lofs : lofs + QB],
                    )
                    cfs.append(t)
                for oc in range(HPC):
                    ps = mmpool.tile([128, QB], f32, tag="mmps", name=f"ops{b}{tbo}{oc}")
                    for a in range(NA):
                        nc.tensor.matmul(
                            ps[:],
                            wo_sb[:, a * 512 + oc * 128 : a * 512 + (oc + 1) * 128],
                            cfs[a // (NA // 2)][:, (a % (NA // 2)) * QB : (a % (NA // 2) + 1) * QB],
                            start=(a == 0), stop=(a == NA - 1),
                        )
                    osb = wpool.tile([128, QB], f32, tag="osb", name=f"ob{b}{tbo}{oc}")
                    nc.vector.tensor_copy(osb[:], ps[:])
                    nc.sync.dma_start(
                        outT[oc * 128 : (oc + 1) * 128, gtok : gtok + QB], osb[:]
                    )

            # ================= emission schedule =================
            for b in range(B):
                kT_sb = kvpool.tile([128, S], bf, tag="kT", name=f"kT{b}")
                v_sb = kvpool.tile([128, S], bf, tag="v", name=f"v{b}")
                for qbi in range(NQB):
                    qts = proj_block(b, qbi, kT_sb, v_sb)
                    attn_block(b, qbi, qts, kT_sb, v_sb)
                    if qbi == 1:
                        allgather(b, 0)
                    if b == 1 and qbi >= 2:
                        oproj_block(0, qbi)  # overlap b0 o_proj with b1 tail
                allgather(b, 1)
            oproj_block(0, 0)
            oproj_block(0, 1)
            for tbo in range(NQB):
                oproj_block(1, tbo)

    nc.compile()
    return nc


_NC = None


def _get_nc():
    global _NC
    if _NC is None:
        _NC = _build_nc()
    return _NC


def _prep_inputs(hidden_states, q_proj_w, k_proj_w, v_proj_w, o_proj_w, position_ids):
    hidden_states = np.asarray(hidden_states, dtype=np.float32)
    # pre-tile: hT[tb, p, a, t] = hidden[tb*QB + t, a*128 + p]
    hT = np.ascontiguousarray(
        hidden_states.reshape(BT // QB, QB, NA, 128).transpose(0, 3, 2, 1)
    ).astype(BF16).reshape((BT // QB) * 128, NA * QB)

    pos = np.asarray(position_ids)[0].astype(np.float32)  # [S]
    inv = 1.0 / (THETA ** (np.arange(0, DH, 2, dtype=np.float32) / DH))  # [64]
    ang = pos[:, None] * inv[None, :]  # [S, 64]
    c = np.cos(ang).T.astype(np.float32)  # [64, S]
    s = np.sin(ang).T.astype(np.float32)
    cost = np.ascontiguousarray(np.concatenate([c, c], axis=0))
    sinm = np.ascontiguousarray(np.concatenate([-s, s], axis=0))

    kj = np.arange(128)[:, None]
    qi = np.arange(QB)[None, :]
    masks = []
    for r in MASK_RS:
        d = (8 - r) * 128 + qi - kj
        masks.append(((d >= 0) & (d < WIN)).astype(np.float32))
    maskt = np.ascontiguousarray(np.concatenate(masks, axis=0)).astype(BF16)

    q_proj_w = np.asarray(q_proj_w, dtype=np.float32)
    k_proj_w = np.asarray(k_proj_w, dtype=np.float32)
    v_proj_w = np.asarray(v_proj_w, dtype=np.float32)
    o_proj_w = np.asarray(o_proj_w, dtype=np.float32)

    def wtile(wT):
        """[HID, D] (hid-major) -> [128, NA*D] matching SBUF layout:
        out[p, a*D+dd] = wT[a*128+p, dd]."""
        dcols = wT.shape[1]
        return np.ascontiguousarray(
            wT.reshape(NA, 128, dcols).transpose(1, 0, 2).reshape(128, NA * dcols)
        ).astype(BF16)

    # maskt: [128, m*QB] with slot m at cols [m*QB, (m+1)*QB)
    maskt = np.ascontiguousarray(
        maskt.reshape(len(MASK_RS), 128, QB).transpose(1, 0, 2).reshape(
            128, len(MASK_RS) * QB
        )
    )

    in_maps = []
    for core in range(NCORES):
        r0q = core * HPC * DH
        r0k = core * DH
        in_maps.append(
            {
                "hiddenT": hT,
                "wq": wtile(q_proj_w[r0q : r0q + HPC * DH, :].T),
                "wk": wtile(k_proj_w[r0k : r0k + DH, :].T),
                "wv": wtile(v_proj_w[r0k : r0k + DH, :].T),
                "wo": wtile(o_proj_w[r0q : r0q + HPC * DH, :].T),
                "cost": cost,
                "sinm": sinm,
                "maskt": maskt,
                "ident": np.eye(128, dtype=np.float32).astype(BF16),
            }
        )
    return in_maps


def run(inputs, trace=False):
    from concourse.bass_utils import run_bass_kernel_spmd

    nc = _get_nc()
    in_maps = _prep_inputs(
        inputs["hidden_states"],
        inputs["q_proj_w"],
        inputs["k_proj_w"],
        inputs["v_proj_w"],
        inputs["o_proj_w"],
        inputs["position_ids"],
    )
    res = run_bass_kernel_spmd(
        nc, in_maps, core_ids=list(range(NCORES)), trace=trace
    )
    out = np.empty((BT, HID), dtype=np.float32)
    for core in range(NCORES):
        o = np.asarray(res.results[core]["outT"], dtype=np.float32)  # [512, BT]
        out[:, core * HPC * DH : (core + 1) * HPC * DH] = o.T
    return out.reshape(B, S, HID), res


def kernel(**inputs):
    out, _ = run(inputs, trace=False)
    return out
